# revision 17
# baseline (speedup 1.0000x reference)
"""Trainium2 Bass kernel for nn_DecoderLayer_70205535421363.

Decoder layer (pre-LN, T5-style RMSNorm, QK-norm attention + gated-silu MLP)
B=2, S=2048, D=2048, H=16, HD=128, F=8192, fp32.

Fused single-launch design: 8 cores = 2 batches x 4 query blocks.
Core c handles batch b=c//4, query/sequence block j=c%4 (512 positions).
Each core computes:
  - K/V projections for the FULL sequence (all 16 heads, in 4 passes of 4
    heads to fit SBUF), with per-position 1/rms(x) computed on device and
    folded into V; per-head K rmsnorm folded into the exp() scale.
  - Q projection + per-head QK-norm for its own 512 queries only.
  - scores/AV with unnormalized-softmax trick (ones column in the AV matmul
    gives denominators), causal mask supplied as per-core data.
  - transposed output projection + residual + on-device RMSNorm.
  - full-weight gated MLP on its [D, 512] slice -> final output slice.
No collectives, no host round-trip: one NEFF execution per layer.
All learned norm scales folded on the host (ln1/ln2 into weights; qln*kln
applied to q_hat on device).
"""
import numpy as np
import ml_dtypes
from contextlib import ExitStack

import jax
import jax.numpy as jnp
from jax.sharding import Mesh, PartitionSpec, NamedSharding
from jax.experimental.shard_map import shard_map

import concourse.bass as bass
import concourse.tile as tile
import concourse.mybir as mybir
from concourse.bass2jax import _bass_exec_p, install_neuronx_cc_hook, partition_id_tensor
from concourse.vector_clock import ScopedClock
from concourse.masks import make_identity

F32 = mybir.dt.float32
F32R = mybir.dt.float32r
BF16 = mybir.dt.bfloat16
AF = mybir.ActivationFunctionType
bf16 = ml_dtypes.bfloat16

B, S, D, H, HD, F = 2, 2048, 2048, 16, 128, 8192
EPS = 1e-6
SB = 512          # seq positions per core (MLP/output shard)
NH = 4            # heads per attention pass
NPASS = H // NH
ST = S // 128
DT = D // 128
FT = F // 128
N_CORES = 8
MASKW = (ST - 1) * 128 + SB   # 2432

MAX_WAITS = 1     # this walrus build allows one sync-wait per instruction


# ---------------------------------------------------------------------------
# Tile workarounds for the 1-sync-wait-per-instruction walrus limit
# ---------------------------------------------------------------------------
class TileContextFixed(tile.TileContext):
    def _drain_and_barrier(self, tick_clock, wait_clock):
        nc = self.nc
        probe = nc.sync.nop(nofuse=True)
        wait_clock.add_sem_waits(probe.ins, ScopedClock({None: tick_clock.global_clock}))
        si = probe.ins.sync_info
        waits = list(si.on_wait) if si is not None else []
        if len(waits) > MAX_WAITS:
            si.on_wait = waits[:MAX_WAITS]
            rest = waits[MAX_WAITS:]
            for i in range(0, len(rest), MAX_WAITS):
                extra = nc.sync.nop(nofuse=True)
                extra.ins.sync_info = mybir.SyncInfo(
                    on_wait=rest[i:i + MAX_WAITS], on_update=[])
        nc.sync.drain()
        nc.all_engine_barrier()
        assert self.sems is not None
        popped = nc._tile_sem_poison_stack.pop()
        assert popped is self._sem_poison
        nc.clear_and_free_semaphores(list(self.sems.allocated().values()))
        nc.all_engine_barrier()


def legalize_waits(nc, max_waits=MAX_WAITS):
    for fn in nc.m.functions:
        for bb in fn.blocks:
            insts = bb.instructions
            new_insts = []
            changed = False
            for inst in insts:
                si = inst.sync_info
                if si is not None and len(si.on_wait) > max_waits:
                    waits = list(si.on_wait)
                    keep = waits[:max_waits]
                    rest = waits[max_waits:]
                    for i in range(0, len(rest), max_waits):
                        nop = mybir.InstNoOp(
                            name=nc.get_next_instruction_name(),
                            engine=inst.engine, ins=[], outs=[])
                        nop.sync_info = mybir.SyncInfo(
                            on_wait=rest[i:i + max_waits], on_update=[])
                        nc.register_instruction(nop)
                        new_insts.append(nop)
                        changed = True
                    si.on_wait = keep
                new_insts.append(inst)
            if changed:
                insts.clear()
                insts.extend(new_insts)


# ---------------------------------------------------------------------------
# The fused decoder-layer kernel (one core's program; SPMD-uniform)
# ---------------------------------------------------------------------------
def build_fused():
    nc = bass.Bass()
    # packed inputs (fewer PJRT params => lower per-launch dispatch cost):
    #   xtq: x[b].T with the core's own 512-column block appended
    #   wqk: stacked wq/wk (ln1-folded), wvo: wv (p-major) + wo stacked,
    #   wi01: stacked wi0/wi1 (ln2-folded), wog: second MLP weight, mask.
    xtq = nc.dram_tensor("xtq", [D, S + SB], F32R, kind="ExternalInput")
    wqk = nc.dram_tensor("wqk", [2 * D, H * HD], F32R, kind="ExternalInput")
    qw = nc.dram_tensor("qw", [128, H * HD], F32, kind="ExternalInput")
    wvo = nc.dram_tensor("wvo", [128, 2, DT, H * HD], BF16, kind="ExternalInput")
    mask = nc.dram_tensor("mask", [128, MASKW], BF16, kind="ExternalInput")
    wi01 = nc.dram_tensor("wi01", [128, 2, FT, D], BF16, kind="ExternalInput")
    wog = nc.dram_tensor("wog", [128, DT, F], BF16, kind="ExternalInput")
    out = nc.dram_tensor("out", [D, SB], F32, kind="ExternalOutput")

    # p-major views for single-DMA loads
    xt_p = xtq.rearrange("(dt p) s -> p dt s", p=128)
    wqk_p = wqk.rearrange("(t dt p) f -> p t dt f", p=128, t=2)
    out_p = out.rearrange("(dt p) q -> p dt q", p=128)

    with TileContextFixed(nc) as tc:
      with ExitStack() as top:
        consts = top.enter_context(tc.tile_pool(name="consts", bufs=1))
        eps_sb = consts.tile([128, 1], F32, name="eps_sb")
        nc.vector.memset(eps_sb, EPS)
        id_f = consts.tile([128, 128], F32, name="id_f")
        make_identity(nc, id_f)
        id_b = consts.tile([128, 128], BF16, name="id_b")
        make_identity(nc, id_b)
        ones_b = consts.tile([128, 128], BF16, name="ones_b")
        nc.vector.memset(ones_b, 1.0)
        mask_sb = consts.tile([128, MASKW], BF16, name="mask_sb")
        nc.sync.dma_start(out=mask_sb, in_=mask[:, :])

        persist = top.enter_context(tc.tile_pool(name="persist", bufs=1))
        # all 16 heads' attention output, transposed: [hd, head, q]
        attnT = persist.tile([128, H, SB], BF16, tag="attnT", name="attnT")
        rvb = persist.tile([128, ST], F32, tag="rvb", name="rvb")

        # ================= attention passes (4 heads each) =================
        for g in range(NPASS):
            gsl = slice(g * NH * HD, (g + 1) * NH * HD)
            with ExitStack() as ph:
                wpool = ph.enter_context(tc.tile_pool(name=f"w{g}", bufs=1))
                wq_sb = wpool.tile([128, DT, NH * HD], F32R, name="wq_sb")
                nc.sync.dma_start(out=wq_sb, in_=wqk_p[:, 0, :, gsl])
                wk_sb = wpool.tile([128, DT, NH * HD], F32R, name="wk_sb")
                nc.sync.dma_start(out=wk_sb, in_=wqk_p[:, 1, :, gsl])
                wv_sb = wpool.tile([128, DT, NH * HD], BF16, name="wv_sb")
                nc.sync.dma_start(out=wv_sb, in_=wvo[:, 0, :, gsl])

                hpool = ph.enter_context(tc.tile_pool(name=f"hd{g}", bufs=1))
                QT = [hpool.tile([128, SB], F32R, tag=f"qt{h}", name=f"qt{h}")
                      for h in range(NH)]
                KT = [hpool.tile([128, S], F32R, tag=f"kt{h}", name=f"kt{h}")
                      for h in range(NH)]
                VA = [hpool.tile([128, ST, 132], BF16, tag=f"va{h}", name=f"va{h}")
                      for h in range(NH)]
                rkb = hpool.tile([128, ST, NH], F32, tag="rkb", name="rkb")
                for h in range(NH):
                    nc.vector.memset(VA[h][:, :, 128:129], 1.0)

                # ---- Q projection + per-head rmsnorm (own 512 queries) ----
                with ExitStack() as qph:
                    qpool = qph.enter_context(tc.tile_pool(name=f"qp{g}", bufs=1))
                    xq_sb = qpool.tile([128, DT, SB], F32R, name="xq_sb")
                    nc.sync.dma_start(out=xq_sb, in_=xt_p[:, :, S:S + SB])
                    qw_sb = qpool.tile([128, NH * HD], F32, name="qw_sb")
                    nc.sync.dma_start(out=qw_sb, in_=qw[:, gsl])
                    qtmp = qph.enter_context(tc.tile_pool(name=f"qt{g}", bufs=2))
                    qsc = qph.enter_context(tc.tile_pool(name=f"qs{g}", bufs=2))
                    psq = qph.enter_context(
                        tc.tile_pool(name=f"pq{g}", bufs=2, space="PSUM"))
                    pst = qph.enter_context(
                        tc.tile_pool(name=f"pt{g}", bufs=2, space="PSUM"))
                    for qs in range(4):
                        q_ps = psq.tile([128, NH * HD], F32, tag="q", name="q_ps")
                        for d in range(DT):
                            nc.tensor.matmul(
                                q_ps, xq_sb[:, d, qs * 128:(qs + 1) * 128],
                                wq_sb[:, d, :], start=(d == 0), stop=(d == DT - 1))
                        sq = qsc.tile([128, HD], F32, tag="sq", name="sq")
                        ssq = qsc.tile([128, NH], F32, tag="ssq", name="ssq")
                        for h in range(NH):
                            sl = slice(h * HD, (h + 1) * HD)
                            nc.scalar.activation(out=sq, in_=q_ps[:, sl],
                                                 func=AF.Square,
                                                 accum_out=ssq[:, h:h + 1])
                        rq = qsc.tile([128, NH], F32, tag="rq", name="rq")
                        nc.scalar.activation(out=rq, in_=ssq, func=AF.Sqrt,
                                             scale=1.0 / HD, bias=eps_sb)
                        nc.vector.reciprocal(rq, rq)
                        qh = qtmp.tile([128, NH * HD], F32, tag="qh", name="qh")
                        for h in range(NH):
                            sl = slice(h * HD, (h + 1) * HD)
                            nc.vector.tensor_scalar_mul(
                                qh[:, sl], q_ps[:, sl], rq[:, h:h + 1])
                        nc.vector.tensor_tensor(out=qh, in0=qh, in1=qw_sb,
                                                op=mybir.AluOpType.mult)
                        qt_ps = pst.tile([128, NH * HD], F32, tag="qt", name="qt_ps")
                        for h in range(NH):
                            sl = slice(h * HD, (h + 1) * HD)
                            nc.tensor.transpose(qt_ps[:, sl], qh[:, sl], id_f)
                            nc.vector.tensor_copy(
                                QT[h][:, qs * 128:(qs + 1) * 128], qt_ps[:, sl])

                # ---- K/V projection over full S (+ rv on pass 0) ----
                with ExitStack() as kph:
                    xpool = kph.enter_context(tc.tile_pool(name=f"xc{g}", bufs=2))
                    xbpool = kph.enter_context(tc.tile_pool(name=f"xb{g}", bufs=2))
                    ktmp = kph.enter_context(tc.tile_pool(name=f"kt{g}", bufs=2))
                    ksc = kph.enter_context(tc.tile_pool(name=f"ks{g}", bufs=3))
                    psk = kph.enter_context(
                        tc.tile_pool(name=f"pk{g}", bufs=2, space="PSUM"))
                    pkt = kph.enter_context(
                        tc.tile_pool(name=f"pkt{g}", bufs=2, space="PSUM"))
                    prv = kph.enter_context(
                        tc.tile_pool(name=f"prv{g}", bufs=1, space="PSUM"))

                    xqueue = []

                    def load_x(st):
                        xcol = xpool.tile([128, DT, 128], F32R, tag="x", name="xcol")
                        nc.sync.dma_start(
                            out=xcol, in_=xt_p[:, :, st * 128:(st + 1) * 128])
                        xqueue.append(xcol)

                    load_x(0)
                    load_x(1)
                    for st in range(ST):
                        if st + 2 < ST:
                            load_x(st + 2)
                        xcol = xqueue.pop(0)
                        k_ps = psk.tile([128, NH * HD], F32, tag="k", name="k_ps")
                        for d in range(DT):
                            nc.tensor.matmul(k_ps, xcol[:, d, :], wk_sb[:, d, :],
                                             start=(d == 0), stop=(d == DT - 1))
                        xcb = xbpool.tile([128, DT, 128], BF16, tag="xb", name="xcb")
                        nc.vector.tensor_copy(xcb, xcol)
                        v_ps = psk.tile([128, NH * HD], F32, tag="v", name="v_ps")
                        for d in range(DT):
                            nc.tensor.matmul(v_ps, xcb[:, d, :], wv_sb[:, d, :],
                                             start=(d == 0), stop=(d == DT - 1))
                        if g == 0:
                            # rv[s] = 1/sqrt(mean_d x^2 + eps) via ones-matmul
                            sqx = ktmp.tile([128, DT, 128], BF16, tag="sqx",
                                            name="sqx")
                            nc.vector.tensor_tensor(out=sqx, in0=xcb, in1=xcb,
                                                    op=mybir.AluOpType.mult)
                            rv_ps = prv.tile([128, 128], F32, tag="rv", name="rv_ps")
                            for d in range(DT):
                                nc.tensor.matmul(rv_ps, ones_b, sqx[:, d, :],
                                                 start=(d == 0), stop=(d == DT - 1))
                            rv_sb = ksc.tile([128, 128], F32, tag="rvs", name="rv_sb")
                            nc.scalar.activation(out=rv_sb, in_=rv_ps, func=AF.Copy)
                            rvt_ps = prv.tile([128, 128], F32, tag="rvt",
                                              name="rvt_ps")
                            nc.tensor.transpose(rvt_ps, rv_sb, id_f)
                            nc.scalar.activation(out=rvb[:, st:st + 1],
                                                 in_=rvt_ps[:, 0:1], func=AF.Sqrt,
                                                 scale=1.0 / D, bias=eps_sb)
                            nc.vector.reciprocal(rvb[:, st:st + 1],
                                                 rvb[:, st:st + 1])
                        ssk = ksc.tile([128, NH], F32, tag="ssk", name="ssk")
                        sqk = ksc.tile([128, HD], F32, tag="sqk", name="sqk")
                        for h in range(NH):
                            sl = slice(h * HD, (h + 1) * HD)
                            nc.scalar.activation(out=VA[h][:, st, 0:128],
                                                 in_=v_ps[:, sl], func=AF.Copy,
                                                 scale=rvb[:, st:st + 1])
                            nc.scalar.activation(out=sqk, in_=k_ps[:, sl],
                                                 func=AF.Square,
                                                 accum_out=ssk[:, h:h + 1])
                        nc.scalar.activation(out=rkb[:, st, :], in_=ssk,
                                             func=AF.Sqrt, scale=1.0 / HD,
                                             bias=eps_sb)
                        nc.vector.reciprocal(rkb[:, st, :], rkb[:, st, :])
                        ksb = ktmp.tile([128, NH * HD], F32, tag="ksb", name="ksb")
                        nc.vector.tensor_copy(ksb, k_ps)
                        kt_ps = pkt.tile([128, NH * HD], F32, tag="ktp", name="kt_ps")
                        for h in range(NH):
                            sl = slice(h * HD, (h + 1) * HD)
                            nc.tensor.transpose(kt_ps[:, sl], ksb[:, sl], id_f)
                            nc.vector.tensor_copy(
                                KT[h][:, st * 128:(st + 1) * 128], kt_ps[:, sl])

                # ---- scores + AV per head ----
                with ExitStack() as sph:
                    ppool = sph.enter_context(tc.tile_pool(name=f"pr{g}", bufs=4))
                    fpool = sph.enter_context(tc.tile_pool(name=f"fn{g}", bufs=4))
                    pslg = sph.enter_context(
                        tc.tile_pool(name=f"pl{g}", bufs=3, space="PSUM"))
                    psav = sph.enter_context(
                        tc.tile_pool(name=f"pa{g}", bufs=1, space="PSUM"))
                    psat = sph.enter_context(
                        tc.tile_pool(name=f"pat{g}", bufs=1, space="PSUM"))
                    for h in range(NH):
                        hh = g * NH + h
                        att_ps = [psav.tile([128, 132], F32, tag=f"att{qs}",
                                            name=f"att{qs}") for qs in range(4)]
                        for kt in range(ST):
                            lg = pslg.tile([128, SB], F32, tag="lg", name="lg")
                            nc.tensor.matmul(
                                lg, KT[h][:, kt * 128:(kt + 1) * 128], QT[h],
                                start=True, stop=True)
                            pr = ppool.tile([128, SB], BF16, tag="pr", name="pr")
                            nc.scalar.activation(out=pr, in_=lg, func=AF.Exp,
                                                 scale=rkb[:, kt, h:h + 1])
                            moff = (ST - 1 - kt) * 128
                            nc.vector.tensor_tensor(
                                out=pr, in0=pr, in1=mask_sb[:, moff:moff + SB],
                                op=mybir.AluOpType.mult)
                            for qs in range(4):
                                nc.tensor.matmul(
                                    att_ps[qs][:, 0:129],
                                    pr[:, qs * 128:(qs + 1) * 128],
                                    VA[h][:, kt, 0:129],
                                    start=(kt == 0), stop=(kt == ST - 1))
                        for qs in range(4):
                            dr = fpool.tile([128, 1], F32, tag="dr", name="dr")
                            nc.vector.reciprocal(dr, att_ps[qs][:, 128:129])
                            asb = fpool.tile([128, 128], BF16, tag="asb", name="asb")
                            nc.vector.tensor_scalar_mul(
                                asb, att_ps[qs][:, 0:128], dr)
                            at_ps = psat.tile([128, 128], BF16, tag="atp",
                                              name="at_ps")
                            nc.tensor.transpose(at_ps, asb, id_b)
                            nc.scalar.activation(
                                out=attnT[:, hh, qs * 128:(qs + 1) * 128],
                                in_=at_ps, func=AF.Copy)

        # ========== output projection + residual + MLP rmsnorm ==========
        persist2 = top.enter_context(tc.tile_pool(name="persist2", bufs=1))
        interT = persist2.tile([128, DT, SB], F32, tag="interT", name="interT")
        hT = persist2.tile([128, DT, SB], BF16, tag="hT", name="hT")
        with ExitStack() as ph:
            wopool = ph.enter_context(tc.tile_pool(name="wop", bufs=1))
            wo_sb = wopool.tile([128, H, D], BF16, name="wo_sb")
            for h in range(H):
                nc.sync.dma_start(out=wo_sb[:, h, :], in_=wvo[:, 1, h, :])
            xq_sb = wopool.tile([128, DT, SB], F32R, name="xq_sb2")
            nc.sync.dma_start(out=xq_sb, in_=xt_p[:, :, S:S + SB])
            sqpool = ph.enter_context(tc.tile_pool(name="sqp", bufs=2))
            pso = ph.enter_context(tc.tile_pool(name="pso", bufs=2, space="PSUM"))
            pss = ph.enter_context(tc.tile_pool(name="pss", bufs=1, space="PSUM"))
            ss_ps = pss.tile([128, SB], F32, tag="ss", name="ss_ps")
            for dt in range(DT):
                o_ps = pso.tile([128, SB], F32, tag="o", name="o_ps")
                for h in range(H):
                    nc.tensor.matmul(
                        o_ps, wo_sb[:, h, dt * 128:(dt + 1) * 128],
                        attnT[:, h, :], start=(h == 0), stop=(h == H - 1))
                nc.vector.tensor_tensor(out=interT[:, dt, :], in0=o_ps,
                                        in1=xq_sb[:, dt, :],
                                        op=mybir.AluOpType.add)
                sqi = sqpool.tile([128, SB], BF16, tag="sqi", name="sqi")
                nc.vector.tensor_tensor(out=sqi, in0=interT[:, dt, :],
                                        in1=interT[:, dt, :],
                                        op=mybir.AluOpType.mult)
                nc.tensor.matmul(ss_ps, ones_b, sqi,
                                 start=(dt == 0), stop=(dt == DT - 1))
            rr = wopool.tile([128, SB], F32, name="rr")
            nc.scalar.activation(out=rr, in_=ss_ps, func=AF.Sqrt,
                                 scale=1.0 / D, bias=eps_sb)
            nc.vector.reciprocal(rr, rr)
            for dt in range(DT):
                nc.vector.tensor_tensor(out=hT[:, dt, :], in0=interT[:, dt, :],
                                        in1=rr, op=mybir.AluOpType.mult)

        # ================= gated MLP on the [*, 512] slice =================
        with ExitStack() as ph:
            gpool = ph.enter_context(tc.tile_pool(name="gp", bufs=1))
            g_sb = gpool.tile([128, FT, SB], BF16, tag="g", name="g_sb")
            wpool = ph.enter_context(tc.tile_pool(name="wmlp", bufs=3))
            tpool = ph.enter_context(tc.tile_pool(name="tmlp", bufs=4))
            psab = ph.enter_context(tc.tile_pool(name="psab", bufs=2, space="PSUM"))

            wqueue = []

            def load_w(ft):
                w0c = wpool.tile([128, DT, 128], BF16, tag="w0", name="w0c")
                w1c = wpool.tile([128, DT, 128], BF16, tag="w1", name="w1c")
                nc.sync.dma_start(out=w0c, in_=wi01[:, 0, ft, :])
                nc.sync.dma_start(out=w1c, in_=wi01[:, 1, ft, :])
                wqueue.append((w0c, w1c))

            load_w(0)
            load_w(1)
            for ft in range(FT):
                if ft + 2 < FT:
                    load_w(ft + 2)
                w0c, w1c = wqueue.pop(0)
                a_ps = psab.tile([128, SB], F32, tag="a", name="a_ps")
                for d in range(DT):
                    nc.tensor.matmul(a_ps, w0c[:, d, :], hT[:, d, :],
                                     start=(d == 0), stop=(d == DT - 1))
                b_ps = psab.tile([128, SB], F32, tag="b", name="b_ps")
                for d in range(DT):
                    nc.tensor.matmul(b_ps, w1c[:, d, :], hT[:, d, :],
                                     start=(d == 0), stop=(d == DT - 1))
                ga = tpool.tile([128, SB], BF16, tag="ga", name="ga")
                nc.scalar.activation(out=ga, in_=a_ps, func=AF.Silu)
                gb = tpool.tile([128, SB], BF16, tag="gb", name="gb")
                nc.vector.tensor_copy(gb, b_ps)
                nc.vector.tensor_tensor(out=g_sb[:, ft, :], in0=ga, in1=gb,
                                        op=mybir.AluOpType.mult)

            # ---- second MLP matmul + final residual, streamed per d tile ----
            w2pool = ph.enter_context(tc.tile_pool(name="w2p", bufs=2))
            opool = ph.enter_context(tc.tile_pool(name="op", bufs=3))
            pso2 = ph.enter_context(tc.tile_pool(name="pso2", bufs=2, space="PSUM"))

            w2queue = []

            def load_w2(dt):
                wc = w2pool.tile([128, F], BF16, tag="w2", name="w2c")
                nc.sync.dma_start(out=wc, in_=wog[:, dt, :])
                w2queue.append(wc)

            load_w2(0)
            load_w2(1)
            for dt in range(DT):
                if dt + 2 < DT:
                    load_w2(dt + 2)
                wc = w2queue.pop(0)
                o_ps = pso2.tile([128, SB], F32, tag="o2", name="o2_ps")
                for ft in range(FT):
                    nc.tensor.matmul(o_ps, wc[:, ft * 128:(ft + 1) * 128],
                                     g_sb[:, ft, :],
                                     start=(ft == 0), stop=(ft == FT - 1))
                fin = opool.tile([128, SB], F32, tag="fin", name="fin")
                nc.vector.tensor_tensor(out=fin, in0=o_ps, in1=interT[:, dt, :],
                                        op=mybir.AluOpType.add)
                nc.sync.dma_start(out=out_p[:, dt, :], in_=fin)
    legalize_waits(nc)
    return nc


# ---------------------------------------------------------------------------
# Persistent-jit SPMD runner (zeros folded into the jit body: 1 dispatch/call)
# ---------------------------------------------------------------------------
class SpmdRunner:
    def __init__(self, nc, n_cores=N_CORES):
        install_neuronx_cc_hook()
        self.nc = nc
        self.n_cores = n_cores
        partition_name = nc.partition_id_tensor.name if nc.partition_id_tensor else None
        in_names, out_names, out_avals = [], [], []
        for alloc in nc.m.functions[0].allocations:
            if not isinstance(alloc, mybir.MemoryLocationSet):
                continue
            name = alloc.memorylocations[0].name
            if alloc.kind == "ExternalInput":
                if name != partition_name:
                    in_names.append(name)
            elif alloc.kind == "ExternalOutput":
                shape = tuple(alloc.tensor_shape)
                dtype = mybir.dt.np(alloc.dtype)
                out_avals.append(jax.core.ShapedArray(shape, dtype))
                out_names.append(name)
        self.in_names, self.out_names = in_names, out_names
        self.out_avals = out_avals
        n_params = len(in_names)
        self.n_params = n_params
        all_in_names = list(in_names) + list(out_names)
        if partition_name is not None:
            all_in_names.append(partition_name)

        def _body(*args):
            operands = list(args)
            if partition_name is not None:
                operands.append(partition_id_tensor())
            outs = _bass_exec_p.bind(
                *operands,
                out_avals=tuple(out_avals),
                in_names=tuple(all_in_names),
                out_names=tuple(out_names),
                lowering_input_output_aliases=(),
                sim_require_finite=True,
                sim_require_nnan=True,
                nc=nc,
            )
            return tuple(outs)

        devices = jax.devices()[:n_cores]
        assert len(devices) >= n_cores, f"need {n_cores} neuron cores"
        self.mesh = Mesh(np.asarray(devices[:n_cores]), ("core",))
        n_outs = len(out_names)
        in_specs = (PartitionSpec("core"),) * (n_params + n_outs)
        out_specs = (PartitionSpec("core"),) * n_outs
        self._fn = jax.jit(
            shard_map(_body, mesh=self.mesh, in_specs=in_specs,
                      out_specs=out_specs, check_rep=False),
            keep_unused=True)
        # Output buffers are fully overwritten by the kernel, so one shared
        # zeros array per output (allocated once, never donated) suffices.
        sh = NamedSharding(self.mesh, PartitionSpec("core"))
        self._zeros = [
            jax.device_put(
                np.zeros((n_cores * a.shape[0], *a.shape[1:]), a.dtype), sh)
            for a in out_avals
        ]

    def concat_inputs(self, in_maps):
        per_core = [[np.asarray(m[name]) for name in self.in_names] for m in in_maps]
        return [np.concatenate([per_core[c][i] for c in range(self.n_cores)], axis=0)
                for i in range(self.n_params)]

    def device_put_inputs(self, concat_in):
        sh = NamedSharding(self.mesh, PartitionSpec("core"))
        return [jax.device_put(a, sh) for a in concat_in]

    def __call__(self, concat_in):
        return self._fn(*(list(concat_in) + self._zeros))

    def split_outputs(self, out_arrs):
        return [
            {name: np.asarray(out_arrs[i]).reshape(
                self.n_cores, *self.out_avals[i].shape)[c]
             for i, name in enumerate(self.out_names)}
            for c in range(self.n_cores)
        ]

    def run(self, in_maps):
        return self.split_outputs(self(self.device_put_inputs(
            self.concat_inputs(in_maps))))


_RUNNERS = {}


def _get_runner():
    if "fused" not in _RUNNERS:
        _RUNNERS["fused"] = SpmdRunner(build_fused())
    return _RUNNERS["fused"]


# ---------------------------------------------------------------------------
# Host-side prep + the public kernel() entry point
# ---------------------------------------------------------------------------
def _fused_in_maps(x, ln1_scale, wq, wk, wv, qln_scale, kln_scale, wo, ln2_scale,
                   wi0, wi1, wout):
    wq_f = (wq * ln1_scale[:, None, None]).reshape(D, H * HD)
    wk_f = (wk * ln1_scale[:, None, None]).reshape(D, H * HD)
    wqk_r = np.ascontiguousarray(
        np.concatenate([wq_f, wk_f], axis=0)).astype(np.float32)
    wv_f = (wv * ln1_scale[:, None, None]).reshape(D, H * HD).astype(bf16)
    # wv p-major: [128(p), DT, H*HD]; wo [H, HD, D] -> [128(hd), H, D];
    # both [128, 16, 2048] -> stack into wvo [128, 2, 16, 2048]
    wv_pm = wv_f.reshape(DT, 128, H * HD).transpose(1, 0, 2)
    wo_r = wo.transpose(1, 0, 2).astype(bf16)
    wvo_r = np.ascontiguousarray(np.stack([wv_pm, wo_r], axis=1))
    qkw = (qln_scale * kln_scale).astype(np.float32)
    qw_tile = np.ascontiguousarray(
        np.broadcast_to(np.tile(qkw, H)[None, :], (128, H * HD))).astype(np.float32)
    # wi [D, F] -> [128(p), FT, D(dt*128+c)] with element (p,ft,dt*128+c) =
    # wi[dt*128+p? no: see below] ... layout: [p, ft, dt, c] = wi[dt*128+c?]
    # We need in0 stationary [128(d), 128(f)] slices: w0c[:, d, fc] =
    # wi[d_global, ft*128+fc] where d_global = d*128 + p.
    # So wi_r[p, ft, d, fc] = wi[d*128+p, ft*128+fc]:
    wi0_f = (wi0 * ln2_scale[:, None]).astype(bf16)
    wi1_f = (wi1 * ln2_scale[:, None]).astype(bf16)
    wi0_r = wi0_f.reshape(DT, 128, FT, 128).transpose(1, 2, 0, 3).reshape(128, FT, D)
    wi1_r = wi1_f.reshape(DT, 128, FT, 128).transpose(1, 2, 0, 3).reshape(128, FT, D)
    wi01_r = np.ascontiguousarray(np.stack([wi0_r, wi1_r], axis=1))
    # wout [F, D] -> wog[p, dt, ft*128+c] = wout[ft*128+p, dt*128+c]
    wog_r = np.ascontiguousarray(
        wout.astype(bf16).reshape(FT, 128, DT, 128).transpose(1, 2, 0, 3)
        .reshape(128, DT, F))

    in_maps = []
    xts = [np.ascontiguousarray(x[b].T) for b in range(B)]       # [D, S]
    for c in range(N_CORES):
        b, j = c // 4, c % 4
        xtb = xts[b]
        # own 512-column block appended so one tensor serves both layouts
        xtqb = np.ascontiguousarray(
            np.concatenate([xtb, xtb[:, j * SB:(j + 1) * SB]], axis=1))
        # mask[p, m] = 1 iff p <= j*SB + m - (ST-1)*128
        m = np.arange(MASKW)[None, :]
        p = np.arange(128)[:, None]
        maskb = (p <= j * SB + m - (ST - 1) * 128).astype(np.float32).astype(bf16)
        in_maps.append({
            "xtq": xtqb,
            "wqk": wqk_r,
            "qw": qw_tile,
            "wvo": wvo_r,
            "mask": maskb,
            "wi01": wi01_r,
            "wog": wog_r,
        })
    return in_maps


def kernel(x, ln1_scale, wq, wk, wv, qln_scale, kln_scale, wo, ln2_scale,
           wi0, wi1, wout):
    x = np.asarray(x, np.float32)
    ln1_scale = np.asarray(ln1_scale, np.float32)
    wq = np.asarray(wq, np.float32)
    wk = np.asarray(wk, np.float32)
    wv = np.asarray(wv, np.float32)
    qln_scale = np.asarray(qln_scale, np.float32)
    kln_scale = np.asarray(kln_scale, np.float32)
    wo = np.asarray(wo, np.float32)
    ln2_scale = np.asarray(ln2_scale, np.float32)
    wi0 = np.asarray(wi0, np.float32)
    wi1 = np.asarray(wi1, np.float32)
    wout = np.asarray(wout, np.float32)

    runner = _get_runner()
    maps = _fused_in_maps(x, ln1_scale, wq, wk, wv, qln_scale, kln_scale, wo,
                          ln2_scale, wi0, wi1, wout)
    res = runner.run(maps)
    out = np.empty((B, S, D), np.float32)
    for c in range(N_CORES):
        b, j = c // 4, c % 4
        out[b, j * SB:(j + 1) * SB, :] = res[c]["out"].T
    return out
